# revision 11
# baseline (speedup 1.0000x reference)
"""Trainium2 Bass kernel for nn_CausalPredictor (bf16-output fast path).

Reference math (per image y = x[b], all f32):
    zd   = dic @ Wz_w.T + Wz_b                          [K, C]
    att  = softmax((y @ Wy_w.T + Wy_b) @ zd.T * s, k)   [L, K]
    z    = (att * prior) @ dic                          [L, D]
    ly   = y @ cs_w[:, :D].T                            [L, C]
    lz   = z @ cs_w[:, D:].T + cs_b                     [L, C]
    out[i*L+j, c] = ly[i, c] + lz[j, c]                 [L*L, C]

Weight-only algebra is folded on the HOST (M, ebias, gb, csyT; see
_host_weights) and y arrives pre-transposed in bf16, so the device graph is
    ep   = exp(M.T @ yT + ebias)              [K, L]   (f32)
    lz   = (ep_chunk.T @ gb) -> num/den + cs_b          [128j, C]
    ly   = in both orientations (lyT for the PE path, ly for the DVE path)
    out[i, jC+c] = ly[i,c] + lz[j,c]  in bf16  (2e-2 gate; bf16 err ~0.5%)

The outer sum is produced by THREE independent paths, balanced so no engine
exceeds the ~65us output-DMA wall:
  * PE path (5 of 8 blocks): fp8e4 DoubleRow matmul against a tiled identity
    rhs (hi+lo fp8 splits of ly/lz), accumulated in f32 PSUM, then cast to
    bf16 on the PSUM->SBUF copy (2/3 on ACT, 1/3 on DVE).
  * DVE path (2 blocks): ob = lz_rep + ly_tiled as a plain bf16 tensor_add,
    where lz_rep is a DMA partition-broadcast of the flattened lz row and
    ly_tiled is a stride-0 access pattern repeating ly[i, :] 128x.
  * GpSimd path (1 block): same tensor_add on the otherwise-idle Pool
    engine; its output DMA rides the scalar ring so the slow block cannot
    stall the sync-ring output FIFO.

Sharding: 8 cores = 4 images x 2 halves of the i dim, no collectives.  The
host hands each core yT with its OWN i-half's columns first; the host
un-permutes the j-halves when assembling.

Ring assignment: sync = small consts + the 7 sync-path output writes,
scalar = yT loads + the GpSimd block's output write, gpsimd = SWDGE
(flatten/broadcast/tiny consts).
"""

import sys

for _p in ("/opt/trn_rl_repo", "/root/.axon_site/_ro/trn_rl_repo"):
    if _p not in sys.path:
        sys.path.append(_p)

import numpy as np

import concourse.bass as bass
from concourse import bacc
import concourse.mybir as mybir
import concourse.tile as tile
from contextlib import ExitStack

B, L, D, K, C = 4, 1024, 1024, 20, 21
SCALE = 1.0 / float(np.sqrt(np.float32(C)))
F32 = mybir.dt.float32
BF16 = mybir.dt.bfloat16
FP8 = mybir.dt.float8e4
HALF_L = L // 2          # 512 rows of i per core
N_IC = HALF_L // 128     # 4 i-chunks of 128 per core
N_DC = D // 128          # 8 chunks along the contraction dim
JC = 512                 # j columns covered by one rhs tile (one j-half)
RHS_W = JC * C           # 10752 free elements per rhs sub-row
Q_N = RHS_W // 512       # 21 matmuls of N=512 per (half, ic)
CHW = 128 * C            # 2688: flattened width of one 128-j lz chunk
DR = mybir.MatmulPerfMode.DoubleRow

# (h, ic) -> producer for the outer-sum block
MODES = {
    (0, 0): "PE", (0, 1): "PE", (0, 2): "PE", (0, 3): "PE",
    (1, 0): "PE", (1, 1): "DVE", (1, 2): "GP", (1, 3): "DVE",
}


def _build_program():
    nc = bacc.Bacc(
        "TRN2",
        target_bir_lowering=False,
        debug=False,
        enable_asserts=False,
        num_devices=8,
    )
    d = {}
    d["yT"] = nc.dram_tensor("yT", [128, N_DC, L], BF16, kind="ExternalInput").ap()
    d["M"] = nc.dram_tensor("M", [128, N_DC, K], BF16, kind="ExternalInput").ap()
    d["csyT"] = nc.dram_tensor("csyT", [128, N_DC, C], BF16, kind="ExternalInput").ap()
    d["gb"] = nc.dram_tensor("gb", [K, C + 1], F32, kind="ExternalInput").ap()
    d["ebias"] = nc.dram_tensor("ebias", [K], F32, kind="ExternalInput").ap()
    d["cs_b"] = nc.dram_tensor("cs_b", [C], F32, kind="ExternalInput").ap()
    d["icorner"] = nc.dram_tensor("icorner", [C, 2, RHS_W], FP8, kind="ExternalInput").ap()
    d["lskel"] = nc.dram_tensor("lskel", [C + 1, 2, HALF_L], FP8, kind="ExternalInput").ap()
    out = nc.dram_tensor("out_loc", [HALF_L, L * C], BF16, kind="ExternalOutput").ap()

    with tile.TileContext(nc) as tc:
        _emit(tc, out, d)
    nc.compile()
    return nc


def _bcast_ap(ap, parts):
    """Partition-broadcast a 1-partition AP across `parts` partitions (DMA only)."""
    return bass.AP(tensor=ap.tensor, offset=ap.offset, ap=[[0, parts]] + list(ap.ap)[1:])


def _rep_ap(ap, reps):
    """Repeat a [P, n] AP's free dim `reps` times via a stride-0 middle dim."""
    aps = list(ap.ap)
    return bass.AP(tensor=ap.tensor, offset=ap.offset, ap=[aps[0], [0, reps]] + aps[1:])


def _emit(tc, out, d):
    nc = tc.nc
    ctx = ExitStack()
    with ctx:
        consts = ctx.enter_context(tc.tile_pool(name="consts", bufs=1))
        outpool = ctx.enter_context(tc.tile_pool(name="outpool", bufs=3))
        small = ctx.enter_context(tc.tile_pool(name="small", bufs=2))
        # PSUM: sm 2 banks + out 3x2 banks = 8.
        sm_ps = ctx.enter_context(tc.tile_pool(name="sm_ps", bufs=2, space="PSUM"))
        out_ps = ctx.enter_context(tc.tile_pool(name="out_ps", bufs=3, space="PSUM"))

        def sm_tile(p, f):
            return sm_ps.tile([p, f], F32, name="sm", tag="sm")

        # ---- loads, spread across all three rings so nothing serializes ----
        # sync ring: lskel first (ly casts WAW-depend on it), then PE-path
        # constants.  scalar ring: the yT halves.  gpsimd ring: tiny consts.
        ly_lhsT = consts.tile([C + 1, 2, HALF_L], FP8, name="ly_lhsT")
        nc.sync.dma_start(out=ly_lhsT, in_=d["lskel"])
        m_sb = consts.tile([128, N_DC, K], BF16, name="m_sb")
        nc.sync.dma_start(out=m_sb, in_=d["M"])
        csy_sb = consts.tile([128, N_DC, C], BF16, name="csy_sb")
        nc.sync.dma_start(out=csy_sb, in_=d["csyT"])
        rhs = [consts.tile([C + 1, 2, RHS_W], FP8, name=f"rhs{h}") for h in range(2)]
        nc.sync.dma_start(out=rhs[0][0:C, :, :], in_=d["icorner"])
        nc.sync.dma_start(out=rhs[1][0:C, :, :], in_=d["icorner"])

        yT = consts.tile([128, N_DC, L], BF16, name="yT")
        nc.scalar.dma_start(out=yT[:, 0:4, 0:JC], in_=d["yT"][:, 0:4, 0:JC])
        nc.scalar.dma_start(out=yT[:, 4:8, 0:JC], in_=d["yT"][:, 4:8, 0:JC])
        nc.scalar.dma_start(out=yT[:, :, JC:L], in_=d["yT"][:, :, JC:L])

        gb = consts.tile([K, C + 1], F32, name="gb")
        nc.gpsimd.dma_start(out=gb, in_=d["gb"])
        ebias = consts.tile([K, 1], F32, name="ebias")
        nc.gpsimd.dma_start(out=ebias, in_=d["ebias"].unsqueeze(1))
        csb_rep = consts.tile([128, C], F32, name="csb_rep")
        nc.gpsimd.dma_start(
            out=csb_rep,
            in_=bass.AP(tensor=d["cs_b"].tensor, offset=d["cs_b"].offset,
                        ap=[[0, 128]] + list(d["cs_b"].ap)),
        )

        # PE warmup: dependency-free bf16 matmuls so the HAM releases the
        # clock gate before the real (latency-critical) matmuls arrive.
        warm = consts.tile([128, 640], BF16, name="warm")
        nc.vector.memset(warm, 0.0)
        for _ in range(8):
            pw = sm_tile(128, 512)
            nc.tensor.matmul(pw, warm[:, 0:128], warm[:, 128:640])

        ep = consts.tile([K, L], F32, name="ep")
        lz_sb = consts.tile([128, N_DC, C], F32, name="lz_sb")
        lz_hi8 = consts.tile([128, N_DC, C], FP8, name="lz_hi8")
        lz_lo8 = consts.tile([128, N_DC, C], FP8, name="lz_lo8")
        lz_rem = consts.tile([128, N_DC, C], F32, name="lz_rem")
        lz_bf = consts.tile([128, N_IC, C], BF16, name="lz_bf")     # h=1 only
        lz_repT = consts.tile([128, RHS_W], BF16, name="lz_repT")   # h=1 bcast
        ly_bf = consts.tile([128, N_IC, C], BF16, name="ly_bf")

        def half_ep(h):
            """ep[:, half] = exp(M.T @ yT_half + ebias)."""
            jsl = slice(h * JC, (h + 1) * JC)
            ps_u = sm_tile(K, JC)
            for dc in range(N_DC):
                nc.tensor.matmul(ps_u, m_sb[:, dc, :], yT[:, dc, jsl],
                                 start=(dc == 0), stop=(dc == N_DC - 1))
            nc.scalar.activation(ep[:, jsl], ps_u,
                                 mybir.ActivationFunctionType.Exp,
                                 bias=ebias, scale=1.0)

        def chunk_lz(lc):
            """lz chunk lc -> fp8 hi/lo splits -> flatten into rhs row 21;
            h=1 chunks additionally get a bf16 flatten + partition bcast."""
            h, lc4 = lc // 4, lc % 4
            csl = slice(lc * 128, (lc + 1) * 128)
            ps_nd = sm_tile(128, C + 1)
            nc.tensor.matmul(ps_nd, ep[:, csl], gb)
            recip = small.tile([128, 1], F32, name="recip", tag="recip")
            nc.vector.reciprocal(recip, ps_nd[:, C : C + 1])
            nc.vector.scalar_tensor_tensor(
                lz_sb[:, lc, :], ps_nd[:, 0:C], recip, csb_rep,
                op0=mybir.AluOpType.mult, op1=mybir.AluOpType.add)
            hi32 = small.tile([128, C], F32, name="hi32", tag="hi32")
            nc.scalar.copy(lz_hi8[:, lc, :], lz_sb[:, lc, :])
            nc.vector.tensor_copy(hi32, lz_hi8[:, lc, :])
            nc.vector.tensor_sub(lz_rem[:, lc, :], lz_sb[:, lc, :], hi32)
            nc.scalar.copy(lz_lo8[:, lc, :], lz_rem[:, lc, :])
            fsl = slice(lc4 * CHW, (lc4 + 1) * CHW)
            nc.gpsimd.dma_start(out=rhs[h][C : C + 1, 0, fsl], in_=lz_hi8[:, lc, :])
            nc.gpsimd.dma_start(out=rhs[h][C : C + 1, 1, fsl], in_=lz_lo8[:, lc, :])
            if h == 1:
                nc.scalar.copy(lz_bf[:, lc4, :], lz_sb[:, lc, :])
                nc.gpsimd.dma_start(out=lz_repT[0:1, fsl], in_=lz_bf[:, lc4, :])

        def chunk_ly(ic):
            """lyT cols ic*128.. -> fp8 hi/lo into ly_lhsT, bf16 into ly_bf."""
            csl = slice(ic * 128, (ic + 1) * 128)
            ps_lyc = sm_tile(C, 128)
            for dc in range(N_DC):
                nc.tensor.matmul(ps_lyc, csy_sb[:, dc, :], yT[:, dc, csl],
                                 start=(dc == 0), stop=(dc == N_DC - 1))
            nc.scalar.copy(ly_lhsT[0:C, 0, csl], ps_lyc)
            hi32 = small.tile([C, 128], F32, name="lyhi32", tag="lyhi32")
            rem = small.tile([C, 128], F32, name="lyrem", tag="lyrem")
            nc.vector.tensor_copy(hi32, ly_lhsT[0:C, 0, csl])
            nc.vector.tensor_sub(rem, ps_lyc, hi32)
            nc.scalar.copy(ly_lhsT[0:C, 1, csl], rem)
            # row-major ly for the DVE/GP direct path
            ps_lyb = sm_tile(128, C)
            for dc in range(N_DC):
                nc.tensor.matmul(ps_lyb, yT[:, dc, csl], csy_sb[:, dc, :],
                                 start=(dc == 0), stop=(dc == N_DC - 1))
            nc.vector.tensor_copy(ly_bf[:, ic, :], ps_lyb)

        cp_cnt = [0]

        def outer_pe(h, ic, ob):
            """PE path: 21 DoubleRow matmuls + PSUM->SBUF cast copies."""
            lhs = ly_lhsT[:, :, ic * 128 : (ic + 1) * 128]
            q = 0
            while q < Q_N:
                nq = min(2, Q_N - q)
                pt = out_ps.tile([128, 1024], F32, name="po", tag="po")
                for t in range(nq):
                    nc.tensor.matmul(pt[:, t * 512 : (t + 1) * 512], lhs,
                                     rhs[h][:, :, (q + t) * 512 : (q + t + 1) * 512],
                                     perf_mode=DR)
                dst = ob[:, q * 512 : (q + nq) * 512]
                if cp_cnt[0] % 3 == 2:
                    nc.vector.tensor_copy(dst, pt[:, 0 : nq * 512])
                else:
                    nc.scalar.copy(dst, pt[:, 0 : nq * 512])
                cp_cnt[0] += 1
                q += nq

        def outer_direct(eng, ic, ob):
            """DVE/GP path: ob = lz_rep + ly[ic] tiled 128x (stride-0 AP)."""
            for g in range(4):
                fsl = slice(g * CHW, (g + 1) * CHW)
                eng.tensor_add(ob[:, fsl], lz_repT[:, fsl],
                               _rep_ap(ly_bf[:, ic, :], 128))

        def outer_sum(h, ic):
            mode = MODES[(h, ic)]
            tag, bufs = ("obg", 1) if mode == "GP" else ("ob", None)
            ob = outpool.tile([128, RHS_W], BF16, name="ob", tag=tag, bufs=bufs)
            if mode == "PE":
                outer_pe(h, ic, ob)
            elif mode == "DVE":
                outer_direct(nc.vector, ic, ob)
            else:
                outer_direct(nc.gpsimd, ic, ob)
            ring = nc.scalar if mode == "GP" else nc.sync
            ring.dma_start(
                out=out[ic * 128 : (ic + 1) * 128,
                        h * RHS_W : (h + 1) * RHS_W],
                in_=ob,
            )

        # h=0 attention + lz + ly, then the outer blocks with h=1's small ops
        # interleaved between copy batches (in-order engine queues).  The two
        # DVE blocks sit mid-stream and last; the GP block computes in the
        # background from early on.
        half_ep(0)
        for lc in range(4):
            chunk_lz(lc)
        for ic in range(N_IC):
            chunk_ly(ic)
        outer_sum(0, 0)
        half_ep(1)
        outer_sum(0, 1)
        for lc in (4, 5, 6, 7):
            chunk_lz(lc)
        # replicate the flattened lz row to all 128 partitions by log2
        # doubling (SBUF->SBUF partition-shift DMAs; a stride-0 partition
        # broadcast is not expressible).  The scalar ring is idle here.
        for k in range(7):
            p = 1 << k
            nc.scalar.dma_start(out=lz_repT[p : 2 * p, :], in_=lz_repT[0:p, :])
        outer_sum(1, 2)   # GP, background
        outer_sum(0, 2)
        outer_sum(1, 1)   # DVE
        outer_sum(0, 3)
        outer_sum(1, 3)   # DVE
        outer_sum(1, 0)


_NC_CACHE = None


def _get_nc():
    global _NC_CACHE
    if _NC_CACHE is None:
        _NC_CACHE = _build_program()
    return _NC_CACHE


def _host_weights(inputs):
    """Fold the weight-only algebra on the host (float64 for headroom)."""
    import ml_dtypes

    dic = np.asarray(inputs["dic"], np.float64)
    prior = np.asarray(inputs["prior"], np.float64)
    wy_w = np.asarray(inputs["Wy_w"], np.float64)
    wy_b = np.asarray(inputs["Wy_b"], np.float64)
    wz_w = np.asarray(inputs["Wz_w"], np.float64)
    wz_b = np.asarray(inputs["Wz_b"], np.float64)
    cs_w = np.asarray(inputs["cs_w"], np.float64)
    cs_b = np.asarray(inputs["cs_b"], np.float32)

    zdts = (wz_w @ dic.T + wz_b[:, None]) * float(SCALE)   # [C, K]
    m = (wy_w.T @ zdts).astype(ml_dtypes.bfloat16)         # [D, K]
    m = np.ascontiguousarray(m.reshape(N_DC, 128, K).transpose(1, 0, 2))
    ebias = (wy_b @ zdts).astype(np.float32)               # [K]
    g = (prior[:, None] * dic) @ cs_w[:, D:].T             # [K, C]
    gb = np.concatenate([g, np.ones((K, 1))], axis=1).astype(np.float32)
    csyT = cs_w[:, :D].T.astype(ml_dtypes.bfloat16)        # [D, C]
    csyT = np.ascontiguousarray(csyT.reshape(N_DC, 128, C).transpose(1, 0, 2))
    icorner = np.zeros((C, 2, RHS_W), ml_dtypes.float8_e4m3)
    for c in range(C):
        icorner[c, :, c::C] = 1.0
    lskel = np.zeros((C + 1, 2, HALF_L), ml_dtypes.float8_e4m3)
    lskel[C, :, :] = 1.0
    return {
        "icorner": icorner,
        "lskel": lskel,
        "M": m,
        "csyT": csyT,
        "gb": np.ascontiguousarray(gb),
        "ebias": np.ascontiguousarray(ebias),
        "cs_b": np.ascontiguousarray(cs_b),
    }


def make_in_maps(inputs):
    import ml_dtypes

    x = np.asarray(inputs["x"], dtype=np.float32)
    w = _host_weights(inputs)
    xT = [np.ascontiguousarray(x[b].T) for b in range(B)]  # [D, L] each
    in_maps = []
    for core in range(8):
        b, ihalf = core % B, core // B
        if ihalf == 0:
            yt = xT[b]
        else:
            yt = np.concatenate([xT[b][:, HALF_L:], xT[b][:, :HALF_L]], axis=1)
        yt = np.ascontiguousarray(
            yt.reshape(N_DC, 128, L).transpose(1, 0, 2).astype(ml_dtypes.bfloat16)
        )
        in_maps.append({"yT": yt, **w})
    return in_maps


def assemble(results):
    out = np.empty((B, L, L, C), dtype=np.float32)
    for core in range(8):
        b, ihalf = core % B, core // B
        # device output: [512 i_local, 2 processed-half, 512 j_local, C];
        # processed half 0 covers real j-half `ihalf`, half 1 the other.
        r = results[core]["out_loc"].reshape(HALF_L, 2, JC, C)
        dst = out[b, ihalf * HALF_L : (ihalf + 1) * HALF_L]
        dst[:, ihalf * JC : (ihalf + 1) * JC] = r[:, 0]
        dst[:, (1 - ihalf) * JC : (2 - ihalf) * JC] = r[:, 1]
    return out.reshape(B, L * L, C)


def _install_trace_support():
    """The agent image's antenv lacks axon_hooks, so boot() skipped NTFF hook
    install. Recreate the module and register the ctypes-based hook; also stub
    the S3 artifact upload (no creds in this container)."""
    import types

    if sys.modules.get("antenv.axon_hooks") is None:
        mod = types.ModuleType("antenv.axon_hooks")
        _hook = [None]
        mod.set_axon_ntff_profile_hook = lambda h: _hook.__setitem__(0, h)
        mod.get_axon_ntff_profile_hook = lambda: _hook[0]
        sys.modules["antenv.axon_hooks"] = mod
        import antenv

        antenv.axon_hooks = mod
    import antenv.axon_hooks as ah

    if ah.get_axon_ntff_profile_hook() is None:
        from trn_agent_boot.trn_boot import _ntff_profile_via_ctypes

        ah.set_axon_ntff_profile_hook(
            _ntff_profile_via_ctypes("/opt/axon/libaxon_pjrt.so")
        )
    import concourse.bass_utils as bu

    bu.upload_artifacts = lambda tmpdir: tmpdir


def run(inputs, trace=False, **kw):
    from concourse.bass_utils import run_bass_kernel_spmd

    if trace:
        _install_trace_support()
    nc = _get_nc()
    res = run_bass_kernel_spmd(
        nc, make_in_maps(inputs), core_ids=list(range(8)), trace=trace, **kw
    )
    return assemble(res.results), res


def kernel(**inputs) -> np.ndarray:
    out, _ = run(inputs, trace=False)
    return out


# revision 12
# speedup vs baseline: 1.2781x; 1.2781x over previous
"""Trainium2 Bass kernel for nn_CausalPredictor (bf16-output fast path).

Reference math (per image y = x[b], all f32):
    zd   = dic @ Wz_w.T + Wz_b                          [K, C]
    att  = softmax((y @ Wy_w.T + Wy_b) @ zd.T * s, k)   [L, K]
    z    = (att * prior) @ dic                          [L, D]
    ly   = y @ cs_w[:, :D].T                            [L, C]
    lz   = z @ cs_w[:, D:].T + cs_b                     [L, C]
    out[i*L+j, c] = ly[i, c] + lz[j, c]                 [L*L, C]

Weight-only algebra is folded on the HOST (M, ebias, gb, csyT; see
_host_weights) and y arrives pre-transposed in bf16, so the device graph is
    ep   = exp(M.T @ yT + ebias)              [K, L]   (f32)
    lz   = (ep_chunk.T @ gb) -> num/den + cs_b          [128j, C]
    ly   = in both orientations (lyT for the PE path, ly for the DVE path)
    out[i, jC+c] = ly[i,c] + lz[j,c]  in bf16  (2e-2 gate; bf16 err ~0.5%)

The outer sum is produced by THREE independent paths, balanced so no engine
exceeds the ~65us output-DMA wall:
  * PE path (5 of 8 blocks): fp8e4 DoubleRow matmul against a tiled identity
    rhs (hi+lo fp8 splits of ly/lz), accumulated in f32 PSUM, then cast to
    bf16 on the PSUM->SBUF copy (2/3 on ACT, 1/3 on DVE).
  * DVE path (2 blocks): ob = lz_rep + ly_tiled as a plain bf16 tensor_add,
    where lz_rep is a DMA partition-broadcast of the flattened lz row and
    ly_tiled is a stride-0 access pattern repeating ly[i, :] 128x.
  * GpSimd path (1 block): same tensor_add on the otherwise-idle Pool
    engine; its output DMA rides the scalar ring so the slow block cannot
    stall the sync-ring output FIFO.

Sharding: 8 cores = 4 images x 2 halves of the i dim, no collectives.  The
host hands each core yT with its OWN i-half's columns first; the host
un-permutes the j-halves when assembling.

Ring assignment: sync = small consts + the 7 sync-path output writes,
scalar = yT loads + the GpSimd block's output write, gpsimd = SWDGE
(flatten/broadcast/tiny consts).
"""

import sys

for _p in ("/opt/trn_rl_repo", "/root/.axon_site/_ro/trn_rl_repo"):
    if _p not in sys.path:
        sys.path.append(_p)

import numpy as np

import concourse.bass as bass
from concourse import bacc
import concourse.mybir as mybir
import concourse.tile as tile
from contextlib import ExitStack

B, L, D, K, C = 4, 1024, 1024, 20, 21
SCALE = 1.0 / float(np.sqrt(np.float32(C)))
F32 = mybir.dt.float32
BF16 = mybir.dt.bfloat16
FP8 = mybir.dt.float8e4
HALF_L = L // 2          # 512 rows of i per core
N_IC = HALF_L // 128     # 4 i-chunks of 128 per core
N_DC = D // 128          # 8 chunks along the contraction dim
JC = 512                 # j columns covered by one rhs tile (one j-half)
RHS_W = JC * C           # 10752 free elements per rhs sub-row
Q_N = RHS_W // 512       # 21 matmuls of N=512 per (half, ic)
CHW = 128 * C            # 2688: flattened width of one 128-j lz chunk
DR = mybir.MatmulPerfMode.DoubleRow



def _build_program():
    nc = bacc.Bacc(
        "TRN2",
        target_bir_lowering=False,
        debug=False,
        enable_asserts=False,
        num_devices=8,
    )
    d = {}
    d["yT"] = nc.dram_tensor("yT", [128, N_DC, L], BF16, kind="ExternalInput").ap()
    d["M"] = nc.dram_tensor("M", [128, N_DC, K], BF16, kind="ExternalInput").ap()
    d["csyT"] = nc.dram_tensor("csyT", [128, N_DC, C], BF16, kind="ExternalInput").ap()
    d["gb"] = nc.dram_tensor("gb", [K, C + 1], F32, kind="ExternalInput").ap()
    d["ebias"] = nc.dram_tensor("ebias", [K], F32, kind="ExternalInput").ap()
    d["cs_b"] = nc.dram_tensor("cs_b", [C], F32, kind="ExternalInput").ap()
    d["icorner"] = nc.dram_tensor("icorner", [C, 2, RHS_W], FP8, kind="ExternalInput").ap()
    d["lskel"] = nc.dram_tensor("lskel", [C + 1, 2, HALF_L], FP8, kind="ExternalInput").ap()
    out = nc.dram_tensor("out_loc", [HALF_L, L * C], BF16, kind="ExternalOutput").ap()

    with tile.TileContext(nc) as tc:
        _emit(tc, out, d)
    nc.compile()
    return nc


def _bcast_ap(ap, parts):
    """Partition-broadcast a 1-partition AP across `parts` partitions (DMA only)."""
    return bass.AP(tensor=ap.tensor, offset=ap.offset, ap=[[0, parts]] + list(ap.ap)[1:])


def _rep_ap(ap, reps):
    """Repeat a [P, n] AP's free dim `reps` times via a stride-0 middle dim."""
    aps = list(ap.ap)
    return bass.AP(tensor=ap.tensor, offset=ap.offset, ap=[aps[0], [0, reps]] + aps[1:])


def _emit(tc, out, d):
    nc = tc.nc
    ctx = ExitStack()
    with ctx:
        consts = ctx.enter_context(tc.tile_pool(name="consts", bufs=1))
        outpool = ctx.enter_context(tc.tile_pool(name="outpool", bufs=3))
        small = ctx.enter_context(tc.tile_pool(name="small", bufs=2))
        # PSUM: sm 2 banks + out 3x2 banks = 8.
        sm_ps = ctx.enter_context(tc.tile_pool(name="sm_ps", bufs=2, space="PSUM"))
        out_ps = ctx.enter_context(tc.tile_pool(name="out_ps", bufs=3, space="PSUM"))

        def sm_tile(p, f):
            return sm_ps.tile([p, f], F32, name="sm", tag="sm")

        # ---- loads, spread across all three rings so nothing serializes ----
        # sync ring: lskel first (ly casts WAW-depend on it), then PE-path
        # constants.  scalar ring: the yT halves.  gpsimd ring: tiny consts.
        ly_lhsT = consts.tile([C + 1, 2, HALF_L], FP8, name="ly_lhsT")
        nc.sync.dma_start(out=ly_lhsT, in_=d["lskel"])
        m_sb = consts.tile([128, N_DC, K], BF16, name="m_sb")
        nc.sync.dma_start(out=m_sb, in_=d["M"])
        csy_sb = consts.tile([128, N_DC, C], BF16, name="csy_sb")
        nc.sync.dma_start(out=csy_sb, in_=d["csyT"])
        rhs = [consts.tile([C + 1, 2, RHS_W], FP8, name=f"rhs{h}") for h in range(2)]
        nc.sync.dma_start(out=rhs[0][0:C, :, :], in_=d["icorner"])
        nc.sync.dma_start(out=rhs[1][0:C, :, :], in_=d["icorner"])

        yT = consts.tile([128, N_DC, L], BF16, name="yT")
        nc.scalar.dma_start(out=yT[:, 0:4, 0:JC], in_=d["yT"][:, 0:4, 0:JC])
        nc.scalar.dma_start(out=yT[:, 4:8, 0:JC], in_=d["yT"][:, 4:8, 0:JC])
        nc.scalar.dma_start(out=yT[:, :, JC:L], in_=d["yT"][:, :, JC:L])

        gb = consts.tile([K, C + 1], F32, name="gb")
        nc.gpsimd.dma_start(out=gb, in_=d["gb"])
        ebias = consts.tile([K, 1], F32, name="ebias")
        nc.gpsimd.dma_start(out=ebias, in_=d["ebias"].unsqueeze(1))
        csb_rep = consts.tile([128, C], F32, name="csb_rep")
        nc.gpsimd.dma_start(
            out=csb_rep,
            in_=bass.AP(tensor=d["cs_b"].tensor, offset=d["cs_b"].offset,
                        ap=[[0, 128]] + list(d["cs_b"].ap)),
        )

        # PE warmup: dependency-free bf16 matmuls so the HAM releases the
        # clock gate before the real (latency-critical) matmuls arrive.
        warm = consts.tile([128, 640], BF16, name="warm")
        nc.vector.memset(warm, 0.0)
        for _ in range(8):
            pw = sm_tile(128, 512)
            nc.tensor.matmul(pw, warm[:, 0:128], warm[:, 128:640])

        ep = consts.tile([K, L], F32, name="ep")
        lz_sb = consts.tile([128, N_DC, C], F32, name="lz_sb")
        lz_hi8 = consts.tile([128, N_DC, C], FP8, name="lz_hi8")
        lz_lo8 = consts.tile([128, N_DC, C], FP8, name="lz_lo8")
        lz_rem = consts.tile([128, N_DC, C], F32, name="lz_rem")

        def half_ep(h):
            """ep[:, half] = exp(M.T @ yT_half + ebias)."""
            jsl = slice(h * JC, (h + 1) * JC)
            ps_u = sm_tile(K, JC)
            for dc in range(N_DC):
                nc.tensor.matmul(ps_u, m_sb[:, dc, :], yT[:, dc, jsl],
                                 start=(dc == 0), stop=(dc == N_DC - 1))
            nc.scalar.activation(ep[:, jsl], ps_u,
                                 mybir.ActivationFunctionType.Exp,
                                 bias=ebias, scale=1.0)

        def chunk_lz(lc):
            """lz chunk lc -> fp8 hi/lo splits -> flatten into rhs row 21;
            h=1 chunks additionally get a bf16 flatten + partition bcast."""
            h, lc4 = lc // 4, lc % 4
            csl = slice(lc * 128, (lc + 1) * 128)
            ps_nd = sm_tile(128, C + 1)
            nc.tensor.matmul(ps_nd, ep[:, csl], gb)
            recip = small.tile([128, 1], F32, name="recip", tag="recip")
            nc.vector.reciprocal(recip, ps_nd[:, C : C + 1])
            nc.vector.scalar_tensor_tensor(
                lz_sb[:, lc, :], ps_nd[:, 0:C], recip, csb_rep,
                op0=mybir.AluOpType.mult, op1=mybir.AluOpType.add)
            hi32 = small.tile([128, C], F32, name="hi32", tag="hi32")
            nc.scalar.copy(lz_hi8[:, lc, :], lz_sb[:, lc, :])
            nc.vector.tensor_copy(hi32, lz_hi8[:, lc, :])
            nc.vector.tensor_sub(lz_rem[:, lc, :], lz_sb[:, lc, :], hi32)
            nc.scalar.copy(lz_lo8[:, lc, :], lz_rem[:, lc, :])
            fsl = slice(lc4 * CHW, (lc4 + 1) * CHW)
            nc.gpsimd.dma_start(out=rhs[h][C : C + 1, 0, fsl], in_=lz_hi8[:, lc, :])
            nc.gpsimd.dma_start(out=rhs[h][C : C + 1, 1, fsl], in_=lz_lo8[:, lc, :])

        def chunk_ly(ic):
            """lyT cols ic*128.. -> fp8 hi/lo into ly_lhsT, bf16 into ly_bf."""
            csl = slice(ic * 128, (ic + 1) * 128)
            ps_lyc = sm_tile(C, 128)
            for dc in range(N_DC):
                nc.tensor.matmul(ps_lyc, csy_sb[:, dc, :], yT[:, dc, csl],
                                 start=(dc == 0), stop=(dc == N_DC - 1))
            nc.scalar.copy(ly_lhsT[0:C, 0, csl], ps_lyc)
            hi32 = small.tile([C, 128], F32, name="lyhi32", tag="lyhi32")
            rem = small.tile([C, 128], F32, name="lyrem", tag="lyrem")
            nc.vector.tensor_copy(hi32, ly_lhsT[0:C, 0, csl])
            nc.vector.tensor_sub(rem, ps_lyc, hi32)
            nc.scalar.copy(ly_lhsT[0:C, 1, csl], rem)

        cp_cnt = [0]

        def outer_pe(h, ic, ob):
            """PE path: 21 DoubleRow matmuls + PSUM->SBUF cast copies."""
            lhs = ly_lhsT[:, :, ic * 128 : (ic + 1) * 128]
            q = 0
            while q < Q_N:
                nq = min(2, Q_N - q)
                pt = out_ps.tile([128, 1024], F32, name="po", tag="po")
                for t in range(nq):
                    nc.tensor.matmul(pt[:, t * 512 : (t + 1) * 512], lhs,
                                     rhs[h][:, :, (q + t) * 512 : (q + t + 1) * 512],
                                     perf_mode=DR)
                dst = ob[:, q * 512 : (q + nq) * 512]
                if cp_cnt[0] % 3 == 2:
                    nc.vector.tensor_copy(dst, pt[:, 0 : nq * 512])
                else:
                    nc.scalar.copy(dst, pt[:, 0 : nq * 512])
                cp_cnt[0] += 1
                q += nq

        def outer_sum(h, ic):
            ob = outpool.tile([128, RHS_W], BF16, name="ob", tag="ob")
            outer_pe(h, ic, ob)
            # two DMAs per block so the sync ring starts draining while the
            # second half of the block is still being copied out of PSUM
            cut = 10 * 512
            nc.sync.dma_start(
                out=out[ic * 128 : (ic + 1) * 128,
                        h * RHS_W : h * RHS_W + cut],
                in_=ob[:, 0:cut],
            )
            nc.sync.dma_start(
                out=out[ic * 128 : (ic + 1) * 128,
                        h * RHS_W + cut : (h + 1) * RHS_W],
                in_=ob[:, cut:RHS_W],
            )

        # h=0 attention + lz + ly, then the outer blocks with h=1's small ops
        # interleaved between copy batches (in-order engine queues).  The two
        # DVE blocks sit mid-stream and last; the GP block computes in the
        # background from early on.
        half_ep(0)
        for lc in range(4):
            chunk_lz(lc)
        for ic in range(N_IC):
            chunk_ly(ic)
        outer_sum(0, 0)
        half_ep(1)
        outer_sum(0, 1)
        for lc in (4, 5, 6, 7):
            chunk_lz(lc)
        outer_sum(0, 2)
        outer_sum(0, 3)
        for ic in range(N_IC):
            outer_sum(1, ic)


_NC_CACHE = None


def _get_nc():
    global _NC_CACHE
    if _NC_CACHE is None:
        _NC_CACHE = _build_program()
    return _NC_CACHE


def _host_weights(inputs):
    """Fold the weight-only algebra on the host (float64 for headroom)."""
    import ml_dtypes

    dic = np.asarray(inputs["dic"], np.float64)
    prior = np.asarray(inputs["prior"], np.float64)
    wy_w = np.asarray(inputs["Wy_w"], np.float64)
    wy_b = np.asarray(inputs["Wy_b"], np.float64)
    wz_w = np.asarray(inputs["Wz_w"], np.float64)
    wz_b = np.asarray(inputs["Wz_b"], np.float64)
    cs_w = np.asarray(inputs["cs_w"], np.float64)
    cs_b = np.asarray(inputs["cs_b"], np.float32)

    zdts = (wz_w @ dic.T + wz_b[:, None]) * float(SCALE)   # [C, K]
    m = (wy_w.T @ zdts).astype(ml_dtypes.bfloat16)         # [D, K]
    m = np.ascontiguousarray(m.reshape(N_DC, 128, K).transpose(1, 0, 2))
    ebias = (wy_b @ zdts).astype(np.float32)               # [K]
    g = (prior[:, None] * dic) @ cs_w[:, D:].T             # [K, C]
    gb = np.concatenate([g, np.ones((K, 1))], axis=1).astype(np.float32)
    csyT = cs_w[:, :D].T.astype(ml_dtypes.bfloat16)        # [D, C]
    csyT = np.ascontiguousarray(csyT.reshape(N_DC, 128, C).transpose(1, 0, 2))
    icorner = np.zeros((C, 2, RHS_W), ml_dtypes.float8_e4m3)
    for c in range(C):
        icorner[c, :, c::C] = 1.0
    lskel = np.zeros((C + 1, 2, HALF_L), ml_dtypes.float8_e4m3)
    lskel[C, :, :] = 1.0
    return {
        "icorner": icorner,
        "lskel": lskel,
        "M": m,
        "csyT": csyT,
        "gb": np.ascontiguousarray(gb),
        "ebias": np.ascontiguousarray(ebias),
        "cs_b": np.ascontiguousarray(cs_b),
    }


def make_in_maps(inputs):
    import ml_dtypes

    x = np.asarray(inputs["x"], dtype=np.float32)
    w = _host_weights(inputs)
    xT = [np.ascontiguousarray(x[b].T) for b in range(B)]  # [D, L] each
    in_maps = []
    for core in range(8):
        b, ihalf = core % B, core // B
        if ihalf == 0:
            yt = xT[b]
        else:
            yt = np.concatenate([xT[b][:, HALF_L:], xT[b][:, :HALF_L]], axis=1)
        yt = np.ascontiguousarray(
            yt.reshape(N_DC, 128, L).transpose(1, 0, 2).astype(ml_dtypes.bfloat16)
        )
        in_maps.append({"yT": yt, **w})
    return in_maps


def assemble(results):
    out = np.empty((B, L, L, C), dtype=np.float32)
    for core in range(8):
        b, ihalf = core % B, core // B
        # device output: [512 i_local, 2 processed-half, 512 j_local, C];
        # processed half 0 covers real j-half `ihalf`, half 1 the other.
        r = results[core]["out_loc"].reshape(HALF_L, 2, JC, C)
        dst = out[b, ihalf * HALF_L : (ihalf + 1) * HALF_L]
        dst[:, ihalf * JC : (ihalf + 1) * JC] = r[:, 0]
        dst[:, (1 - ihalf) * JC : (2 - ihalf) * JC] = r[:, 1]
    return out.reshape(B, L * L, C)


def _install_trace_support():
    """The agent image's antenv lacks axon_hooks, so boot() skipped NTFF hook
    install. Recreate the module and register the ctypes-based hook; also stub
    the S3 artifact upload (no creds in this container)."""
    import types

    if sys.modules.get("antenv.axon_hooks") is None:
        mod = types.ModuleType("antenv.axon_hooks")
        _hook = [None]
        mod.set_axon_ntff_profile_hook = lambda h: _hook.__setitem__(0, h)
        mod.get_axon_ntff_profile_hook = lambda: _hook[0]
        sys.modules["antenv.axon_hooks"] = mod
        import antenv

        antenv.axon_hooks = mod
    import antenv.axon_hooks as ah

    if ah.get_axon_ntff_profile_hook() is None:
        from trn_agent_boot.trn_boot import _ntff_profile_via_ctypes

        ah.set_axon_ntff_profile_hook(
            _ntff_profile_via_ctypes("/opt/axon/libaxon_pjrt.so")
        )
    import concourse.bass_utils as bu

    bu.upload_artifacts = lambda tmpdir: tmpdir


def run(inputs, trace=False, **kw):
    from concourse.bass_utils import run_bass_kernel_spmd

    if trace:
        _install_trace_support()
    nc = _get_nc()
    res = run_bass_kernel_spmd(
        nc, make_in_maps(inputs), core_ids=list(range(8)), trace=trace, **kw
    )
    return assemble(res.results), res


def kernel(**inputs) -> np.ndarray:
    out, _ = run(inputs, trace=False)
    return out
